# revision 37
# baseline (speedup 1.0000x reference)
"""Trainium2 Bass kernel for BertSelfAttention with relative_key_query position
embeddings (fused band/score pipeline, fp8 bands, DoubleRow transposes).

Problem shape: B=8, L=1024, H=1024 (16 heads x 64), MAX_POS=1024.
Sharding: data-parallel over batch -- core b computes batch element b fully.

Math (per batch, per head):
    q = x @ Wq.T + bq ; k, v likewise
    S[l,r] = (q[l]@k[r] + q[l]@de[l-r+1023] + k[r]@de[l-r+1023]) / 8
    P = softmax(S, axis=r);  ctx[l,:] = P[l,:] @ v

Kernel formulation (transposed scores S^T[r,l]):
    - host pre-transposes: xT[j,l], WqT/8, WkT, WvT, de tables; projections
      and QK run in bf16 (PSUM accumulates f32).
    - Toeplitz position terms via banded outer-product matrices in DRAM with
      column-reversed band layout, re-read with a stride-trick AP that
      realizes the per-row diagonal shift. Band blocks pack a head PAIR side
      by side ([128, 2304] fp8e4m3, x64 scaled); partition stride 2303 keeps
      the skew per head.
    - k-term skew tiles land directly as kposT[r',l] and are added into the
      score PSUM by a matmul against a 1/64-scaled identity; q-term tiles
      land as qpos[l',r] and are transposed into the score PSUM two
      128-chunks at a time by fp8 DoubleRow matmuls against a block identity
      ([128, 2, 256], eye/64 blocks) -- out = aq(lb)^T | aq(lb+1)^T.
    - phase B (bands) and phase C (scores/softmax/AV) are software-pipelined
      across head pairs: each emission "round" zips 8 band blocks of pair p
      with 16 rb score units of pair p-1; k-term tiles are loaded
      just-in-time ~3 blocks ahead; scores use chunked [128,512] PSUM tiles
      with per-chunk exp for fine-grained PSUM recycling.
    - softmax without max subtraction (logits bounded); denominator via an
      appended ones-column on v so Z comes out of the AV matmul for free.
    - output produced transposed (ctx*Z | Z rows); host divides+transposes.
"""

import sys

sys.path.insert(0, "/opt/trn_rl_repo")

import numpy as np

import concourse.bass as bass
import concourse.mybir as mybir
import concourse.tile as tile
from concourse import bacc
from concourse.bass_utils import run_bass_kernel_spmd

F32 = mybir.dt.float32
F32R = mybir.dt.float32r
BF16 = mybir.dt.bfloat16
FP8 = mybir.dt.float8e4
BF16_NP = mybir.dt.np(BF16)
FP8_NP = mybir.dt.np(FP8)

B = 8
L = 1024
H = 1024
NH = 16
HD = 64
NB = L // 128
NPAIR = NH // 2
BPITCH = 1152          # per-head band pitch
PPITCH = 2 * BPITCH    # pair-packed block pitch (2304)
BSCALE = 64.0          # band store scale (fp8-friendly)
INV_BSCALE = 1.0 / BSCALE
CHUNKS = [(0, 512), (512, 512), (1024, 128)]

TRACE = False
LAST_RESULTS = None

_CACHE = {}


def _enable_ldw_opt():
    # --enable-ldw-opt=true fails codegen on this kernel's f32r LDWEIGHTS
    # ("InstLdweights is not compatible with LDW optimization"); keep the
    # compiler default.
    return


def _emit(nc, tc, ctx, tensors):
    import contextlib

    xT = tensors["xT"]
    wqT8 = tensors["wqT8"]
    wkT = tensors["wkT"]
    wvT = tensors["wvT"]
    bq8 = tensors["bq8"]
    bk = tensors["bk"]
    bv = tensors["bv"]
    det8 = tensors["det8"]      # de.T / 8     [64, 2048] (k-side band rhs)
    detrev = tensors["detrev"]  # de[::-1].T   [64, 2048] (q-side band rhs)
    outTa = tensors["outTa"]

    ACC = mybir.AluOpType
    AF = mybir.ActivationFunctionType

    # ---------------- persistent pools ----------------
    persist = ctx.enter_context(tc.tile_pool(name="persist", bufs=1))
    qT8_sb = [persist.tile([128, L], BF16, tag=f"qT8_{t}", name=f"qT8_{t}")
              for t in range(NB)]
    kT_sb = [persist.tile([128, L], BF16, tag=f"kT_{t}", name=f"kT_{t}")
             for t in range(NB)]
    vaug_sb = [persist.tile([128, NH * (HD + 1)], BF16, tag=f"vaug_{t}",
                            name=f"vaug_{t}") for t in range(NB)]
    bias_sb = persist.tile([128, 2 * NB], F32, tag="bias")
    det8_sb = persist.tile([128, 2048], BF16, tag="det8")
    detrev_sb = persist.tile([128, 2048], BF16, tag="detrev")

    nc.sync.dma_start(
        out=bias_sb[:, 0:NB],
        in_=bass.AP(tensor=bq8.tensor, offset=0, ap=[[1, 128], [128, NB]]),
    )
    nc.sync.dma_start(
        out=bias_sb[:, NB:2 * NB],
        in_=bass.AP(tensor=bk.tensor, offset=0, ap=[[1, 128], [128, NB]]),
    )
    # de tables replicated on both partition halves (for head-pair packing)
    nc.sync.dma_start(out=det8_sb[0:64, :], in_=det8[:, :])
    nc.sync.dma_start(out=det8_sb[64:128, :], in_=det8[:, :])
    nc.sync.dma_start(out=detrev_sb[0:64, :], in_=detrev[:, :])
    nc.sync.dma_start(out=detrev_sb[64:128, :], in_=detrev[:, :])

    ident2 = tensors["ident2"]
    # block identity for DoubleRow transposes: [128, 2, 256] fp8, eye/64 at
    # [:, 0, 0:128] and [:, 1, 128:256]
    ident2_sb = persist.tile([128, 512], FP8, tag="ident2")
    nc.sync.dma_start(out=ident2_sb, in_=ident2[:, :])
    ident2_ap = bass.AP(tensor=ident2_sb.tensor, offset=ident2_sb.offset,
                        ap=[ident2_sb.ap[0], [256, 2], [1, 256]])

    # DRAM scratch for position bands (pair-packed, column-reversed layout)
    dram = ctx.enter_context(tc.tile_pool(name="dramsc", bufs=1, space="DRAM"))
    aq_band = dram.tile([NPAIR, NB, 128, PPITCH], FP8, tag="aq_band")
    ak_band = dram.tile([NPAIR, NB, 128, PPITCH], FP8, tag="ak_band")

    # ---------------- phase A: projections ----------------
    with contextlib.ExitStack() as phase_a:
        xp = phase_a.enter_context(tc.tile_pool(name="xT", bufs=1))
        xT_sb = [xp.tile([128, L], BF16, tag=f"xT_{t}", name=f"xT_{t}")
                 for t in range(NB)]
        for t in range(NB):
            nc.sync.dma_start(out=xT_sb[t], in_=xT[t * 128:(t + 1) * 128, :])
        bv_sb = xp.tile([128, H], F32, tag="bv")
        nc.sync.dma_start(out=bv_sb, in_=bass.AP(tensor=bv.tensor, offset=0,
                                                 ap=[[0, 128], [1, H]]))

        wp = phase_a.enter_context(tc.tile_pool(name="w", bufs=24))
        pp = phase_a.enter_context(
            tc.tile_pool(name="projps", bufs=3, space="PSUM"))
        for wi, (wten, dst, bias_col) in enumerate(
            [(wqT8, qT8_sb, 0), (wkT, kT_sb, NB)]
        ):
            w_sb = [wp.tile([128, H], BF16, tag="wtile", name="wtile")
                    for _ in range(NB)]
            for jt in range(NB):
                nc.sync.dma_start(out=w_sb[jt],
                                  in_=wten[jt * 128:(jt + 1) * 128, :])
            for ib in range(NB):
                ps = pp.tile([128, L], F32, tag="projps")
                for jt in range(NB):
                    for lc in range(2):
                        nc.tensor.matmul(
                            ps[:, lc * 512:(lc + 1) * 512],
                            lhsT=w_sb[jt][:, ib * 128:(ib + 1) * 128],
                            rhs=xT_sb[jt][:, lc * 512:(lc + 1) * 512],
                            start=(jt == 0),
                            stop=(jt == NB - 1),
                        )
                nc.scalar.activation(
                    out=dst[ib],
                    in_=ps,
                    func=AF.Identity,
                    bias=bias_sb[:, bias_col + ib: bias_col + ib + 1],
                    scale=1.0,
                )

        # V natural [r, i] with ones column per head
        w_sb = [wp.tile([128, H], BF16, tag="wtile", name="wtile")
                for _ in range(NB)]
        for jt in range(NB):
            nc.sync.dma_start(out=w_sb[jt],
                              in_=wvT[jt * 128:(jt + 1) * 128, :])
        for rb in range(NB):
            nc.vector.memset(vaug_sb[rb], 1.0)
            ps = pp.tile([128, H], F32, tag="projps")
            for jt in range(NB):
                for ic in range(2):
                    nc.tensor.matmul(
                        ps[:, ic * 512:(ic + 1) * 512],
                        lhsT=xT_sb[jt][:, rb * 128:(rb + 1) * 128],
                        rhs=w_sb[jt][:, ic * 512:(ic + 1) * 512],
                        start=(jt == 0),
                        stop=(jt == NB - 1),
                    )
            for h in range(NH):
                nc.vector.tensor_tensor(
                    out=vaug_sb[rb][:, h * (HD + 1): h * (HD + 1) + HD],
                    in0=ps[:, h * HD:(h + 1) * HD],
                    in1=bv_sb[:, h * HD:(h + 1) * HD],
                    op=ACC.add,
                )

    # ---------------- fused phase B/C pools ----------------
    bandps = ctx.enter_context(tc.tile_pool(name="bandps", bufs=3,
                                            space="PSUM"))
    sps = ctx.enter_context(tc.tile_pool(name="sps", bufs=3, space="PSUM"))
    ctxps = ctx.enter_context(tc.tile_pool(name="ctxps", bufs=1, space="PSUM"))
    stq = ctx.enter_context(tc.tile_pool(name="stq", bufs=2))
    stk = ctx.enter_context(tc.tile_pool(name="stk", bufs=2))
    aqp = ctx.enter_context(tc.tile_pool(name="aqp", bufs=4))
    kpb = ctx.enter_context(tc.tile_pool(name="kpb", bufs=8))
    prb = ctx.enter_context(tc.tile_pool(name="prb", bufs=3))
    fin = ctx.enter_context(tc.tile_pool(name="fin", bufs=2))

    swp = ctx.enter_context(tc.tile_pool(name="swp", bufs=4))
    swap_tiles = {}

    def emit_swap(pr):
        # partition-swapped copies: rows 64:128 <- rows 0:64 and vice versa,
        # so QK lc0/lc1 can run on disjoint PE row strips concurrently
        qx = swp.tile([128, L], BF16, tag="swp", name="swp")
        kx = swp.tile([128, L], BF16, tag="swp", name="swp")
        for dst, srct in ((qx, qT8_sb[pr]), (kx, kT_sb[pr])):
            nc.sync.dma_start(out=dst[64:128, :], in_=srct[0:64, :])
            nc.sync.dma_start(out=dst[0:64, :], in_=srct[64:128, :])
        swap_tiles[pr] = (qx, kx)

    evict_ctr = [0]

    def evict(out, in_):
        # rotate PSUM evictions: 1/4 ACT, 3/4 DVE (DVE has no STT anymore)
        if evict_ctr[0] % 3 == 0:
            nc.scalar.activation(out=out, in_=in_, func=AF.Copy, scale=BSCALE)
        else:
            nc.vector.tensor_scalar_mul(out, in_, BSCALE)
        evict_ctr[0] += 1

    def gen_B(pr):
        """8 units, blk-major: q-band then k-band outer-products per block."""
        for src_sb, de_sb, band, stpool, stdt in ():
            pass
        for blk in range(NB):
            w0 = 896 - 128 * blk
            for src_sb, de_sb, band, stpool, stdt in (
                (qT8_sb[pr], detrev_sb, aq_band, stq, FP8),
                (kT_sb[pr], det8_sb, ak_band, stk, FP8),
            ):
                stage = stpool.tile([128, PPITCH], stdt, tag="st", name="st")
                for (c0, cw) in CHUNKS:
                    ps_lo = bandps.tile([128, 512], F32, tag="bps",
                                        name="bps")
                    ps_hi = bandps.tile([128, 512], F32, tag="bps",
                                        name="bps")
                    nc.tensor.matmul(
                        ps_lo[:, 0:cw],
                        lhsT=src_sb[0:64, blk * 128:(blk + 1) * 128],
                        rhs=de_sb[0:64, w0 + c0: w0 + c0 + cw],
                        start=True, stop=True, skip_group_check=True,
                    )
                    nc.tensor.matmul(
                        ps_hi[:, 0:cw],
                        lhsT=src_sb[64:128, blk * 128:(blk + 1) * 128],
                        rhs=de_sb[64:128, w0 + c0: w0 + c0 + cw],
                        start=True, stop=True, skip_group_check=True,
                    )
                    evict(stage[:, c0:c0 + cw], ps_lo[:, 0:cw])
                    evict(stage[:, BPITCH + c0:BPITCH + c0 + cw],
                          ps_hi[:, 0:cw])
                nc.sync.dma_start(out=band[pr, blk], in_=stage)
            yield

    def skew_ap(band, pr, blk, hcol):
        base = band[pr, blk, :, :]
        return bass.AP(
            tensor=base.tensor,
            offset=base.offset + hcol * BPITCH + 127,
            ap=[[PPITCH - 1, 128], [1, L]],
        )

    aq_tiles = {}    # h -> [128, NB*L] fp8 skew tiles (qpos[l', r] per lb)
    kpt_tiles = {}   # (h, rb) -> k-term skew tile

    def emit_reads(pr, blk):
        """After B(pr) wrote block blk: q-skew reads for both heads."""
        h0, h1 = 2 * pr, 2 * pr + 1
        if blk == 0:
            aq_tiles[h0] = aqp.tile([128, NB * L], FP8, tag="aqp", name="aqp")
            aq_tiles[h1] = aqp.tile([128, NB * L], FP8, tag="aqp", name="aqp")
        for hcol in (0, 1):
            t = aq_tiles[2 * pr + hcol]
            nc.sync.dma_start(out=t[:, blk * L:(blk + 1) * L],
                              in_=skew_ap(aq_band, pr, blk, hcol))

    def kpt_loads_for(rnd, blk):
        """JIT schedule: load (pair, h, rb) k-term tiles ~3 blks ahead."""
        out = []
        if blk == 0:
            out = [(rnd - 1, 0, 6), (rnd - 1, 0, 7)]
        elif blk <= 4:
            out = [(rnd - 1, 1, 2 * (blk - 1)), (rnd - 1, 1, 2 * blk - 1)]
        else:
            out = [(rnd, 0, 2 * (blk - 5)), (rnd, 0, 2 * blk - 9)]
        return [(p, h, rb) for (p, h, rb) in out if 0 <= p < NPAIR]

    def emit_kpt_loads(rnd, blk):
        for (p, hcol, rb) in kpt_loads_for(rnd, blk):
            t = kpb.tile([128, L], FP8, tag="kpt", name="kpt")
            nc.sync.dma_start(out=t, in_=skew_ap(ak_band, p, rb, hcol))
            kpt_tiles[(2 * p + hcol, rb)] = t

    def emit_av(h, ctx_ps, pv_rb, pv_p):
        for lc in range(2):
            nc.tensor.matmul(
                ctx_ps[:, lc * 512:(lc + 1) * 512],
                lhsT=vaug_sb[pv_rb][:, h * (HD + 1):(h + 1) * (HD + 1)],
                rhs=pv_p[:, lc * 512:(lc + 1) * 512],
                start=(pv_rb == 0), stop=(pv_rb == NB - 1),
                skip_group_check=True,
            )

    def gen_C(pr):
        """16 units: 8 rb units per head; AV software-pipelined by one rb."""
        for h in (2 * pr, 2 * pr + 1):
            hrow = (h % 2) * 64
            aq = aq_tiles[h]
            ctx_ps = ctxps.tile([HD + 1, L], F32, tag="ctxps", name="ctxps")
            prev = None  # (odd rb, p_pair tile)
            qx_sb, kx_sb = swap_tiles[pr]
            xrow = 64 - hrow
            for rb in range(NB):
                kpt = kpt_tiles.pop((h, rb))
                p_sb = prb.tile([128, L], BF16, tag="p", name="p")
                # paired QK: lc0 on this head's home strip, lc1 on the other
                # strip via the partition-swapped copies -- they run
                # concurrently on disjoint PE row strips
                s_chunks = [sps.tile([128, 512], F32, tag="sps", name="sps")
                            for _ in range(2)]
                nc.tensor.matmul(
                    s_chunks[0],
                    lhsT=kT_sb[pr][hrow:hrow + 64, rb * 128:(rb + 1) * 128],
                    rhs=qT8_sb[pr][hrow:hrow + 64, 0:512],
                    start=True, stop=False, skip_group_check=True,
                )
                nc.tensor.matmul(
                    s_chunks[1],
                    lhsT=kx_sb[xrow:xrow + 64, rb * 128:(rb + 1) * 128],
                    rhs=qx_sb[xrow:xrow + 64, 512:1024],
                    start=True, stop=False, skip_group_check=True,
                )
                for lc in range(2):
                    s_ps = s_chunks[lc]
                    for j in range(2):
                        lb = lc * 4 + 2 * j
                        # transpose TWO q-term chunks into score PSUM via the
                        # (1/64-scaled) block identity in fp8 DoubleRow mode:
                        # out[:, 256] = aq(lb)^T | aq(lb+1)^T
                        lhsT = bass.AP(
                            tensor=aq.tensor,
                            offset=aq.offset + lb * L + rb * 128,
                            ap=[aq.ap[0], [L, 2], [1, 128]],
                        )
                        nc.tensor.matmul(
                            s_ps[:, 256 * j:256 * (j + 1)],
                            lhsT=lhsT,
                            rhs=ident2_ap,
                            start=False, stop=(j == 1),
                            perf_mode=mybir.MatmulPerfMode.DoubleRow,
                            skip_group_check=True,
                        )
                    # k-term on DVE: tensor engine is the bottleneck
                    nc.vector.scalar_tensor_tensor(
                        out=s_ps, in0=kpt[:, lc * 512:(lc + 1) * 512],
                        scalar=INV_BSCALE, in1=s_ps,
                        op0=ACC.mult, op1=ACC.add,
                    )
                    nc.scalar.activation(out=p_sb[:, lc * 512:(lc + 1) * 512],
                                         in_=s_ps, func=AF.Exp)
                if prev is not None:
                    pv_rb, pv_p = prev
                    emit_av(h, ctx_ps, pv_rb, pv_p)
                    prev = None
                prev = (rb, p_sb)
                yield
            # drain AV(rb=7) + finalize head
            pv_rb, pv_p = prev
            emit_av(h, ctx_ps, pv_rb, pv_p)
            o_sb = fin.tile([HD + 1, L], F32, tag="osb", name="osb")
            nc.vector.tensor_copy(out=o_sb, in_=ctx_ps)
            nc.sync.dma_start(
                out=outTa[h * (HD + 1):(h + 1) * (HD + 1), :], in_=o_sb)

    # ---------------- pipeline driver ----------------
    # round rnd: B(rnd) band production + per-block reads, zipped with
    # C(rnd-1); round NPAIR is the drain (no band work).
    emit_swap(0)
    for rnd in range(NPAIR + 1):
        bg = gen_B(rnd) if rnd < NPAIR else None
        cg = gen_C(rnd - 1) if rnd >= 1 else None
        if 0 < rnd < NPAIR:
            emit_swap(rnd)
        for blk in range(NB):
            if bg is not None:
                next(bg)
            emit_kpt_loads(rnd, blk)
            if bg is not None:
                emit_reads(rnd, blk)
            if cg is not None:
                next(cg)
                next(cg)
        if cg is not None:
            for _ in cg:   # emit the tail (last AV + finalize of odd head)
                pass


def build_nc():
    if "nc" in _CACHE:
        return _CACHE["nc"]
    import contextlib
    _enable_ldw_opt()

    nc = bacc.Bacc("TRN2", target_bir_lowering=False, debug=False)
    tensors = {
        "xT": nc.dram_tensor("xT", [H, L], BF16, kind="ExternalInput").ap(),
        "wqT8": nc.dram_tensor("wqT8", [H, H], BF16,
                               kind="ExternalInput").ap(),
        "wkT": nc.dram_tensor("wkT", [H, H], BF16, kind="ExternalInput").ap(),
        "wvT": nc.dram_tensor("wvT", [H, H], BF16, kind="ExternalInput").ap(),
        "bq8": nc.dram_tensor("bq8", [H], F32, kind="ExternalInput").ap(),
        "bk": nc.dram_tensor("bk", [H], F32, kind="ExternalInput").ap(),
        "bv": nc.dram_tensor("bv", [H], F32, kind="ExternalInput").ap(),
        "det8": nc.dram_tensor("det8", [HD, 2048], BF16,
                               kind="ExternalInput").ap(),
        "detrev": nc.dram_tensor("detrev", [HD, 2048], BF16,
                                 kind="ExternalInput").ap(),
        "ident2": nc.dram_tensor("ident2", [128, 512], FP8,
                                 kind="ExternalInput").ap(),
        "outTa": nc.dram_tensor("outTa", [NH * (HD + 1), L], F32,
                                kind="ExternalOutput").ap(),
    }
    with contextlib.ExitStack() as ctx:
        tc = ctx.enter_context(tile.TileContext(nc))
        _emit(nc, tc, ctx, tensors)
    nc.compile()
    _CACHE["nc"] = nc
    return nc


def _host_inputs(hidden_states, attention_mask, Wq, bq, Wk, bk, Wv, bv,
                 dist_emb):
    f32 = np.float32
    de = np.ascontiguousarray(dist_emb, dtype=f32)
    pad = np.zeros((HD, 1), np.float32)
    det8 = np.ascontiguousarray(
        np.concatenate([de.T / 8.0, pad], axis=1)).astype(BF16_NP)
    detrev = np.ascontiguousarray(
        np.concatenate([de[::-1].T, pad], axis=1)).astype(BF16_NP)
    ident2 = np.zeros((128, 512), np.float32)
    ident2[np.arange(128), np.arange(128)] = 1.0 / BSCALE
    ident2[np.arange(128), 384 + np.arange(128)] = 1.0 / BSCALE
    base = {
        "wqT8": np.ascontiguousarray(Wq.astype(f32).T / 8.0).astype(BF16_NP),
        "wkT": np.ascontiguousarray(Wk.astype(f32).T).astype(BF16_NP),
        "wvT": np.ascontiguousarray(Wv.astype(f32).T).astype(BF16_NP),
        "bq8": np.ascontiguousarray(bq, dtype=f32) / 8.0,
        "bk": np.ascontiguousarray(bk, dtype=f32),
        "bv": np.ascontiguousarray(bv, dtype=f32),
        "det8": det8, "detrev": detrev,
        "ident2": ident2.astype(FP8_NP),
    }
    in_maps = []
    for b in range(B):
        m = dict(base)
        m["xT"] = np.ascontiguousarray(
            hidden_states[b].astype(f32).T).astype(BF16_NP)
        in_maps.append(m)
    return in_maps


def kernel(**inputs):
    global LAST_RESULTS
    nc = build_nc()
    in_maps = _host_inputs(**{k: np.asarray(v) for k, v in inputs.items()})
    res = run_bass_kernel_spmd(nc, in_maps, core_ids=list(range(B)),
                               trace=TRACE)
    LAST_RESULTS = res
    out = np.empty((B, L, H), np.float32)
    for b in range(B):
        a = res.results[b]["outTa"].reshape(NH, HD + 1, L)
        ctx = a[:, :HD, :] / a[:, HD:HD + 1, :]      # [NH, HD, L]
        out[b] = ctx.transpose(2, 0, 1).reshape(L, H)
    return out


if __name__ == "__main__":
    rng = np.random.default_rng(0)
    demo = {
        "hidden_states": rng.standard_normal((B, L, H), dtype=np.float32),
        "attention_mask": np.zeros((B, 1, 1, L), np.float32),
        "Wq": rng.standard_normal((H, H), dtype=np.float32) * 0.02,
        "bq": np.zeros(H, np.float32),
        "Wk": rng.standard_normal((H, H), dtype=np.float32) * 0.02,
        "bk": np.zeros(H, np.float32),
        "Wv": rng.standard_normal((H, H), dtype=np.float32) * 0.02,
        "bv": np.zeros(H, np.float32),
        "dist_emb": rng.standard_normal((2047, HD), dtype=np.float32) * 0.02,
    }
    out = kernel(**demo)
    print(out.shape, out.dtype)


# revision 38
# speedup vs baseline: 1.2216x; 1.2216x over previous
"""Trainium2 Bass kernel for BertSelfAttention with relative_key_query position
embeddings (fused band/score pipeline, fp8 bands, DoubleRow transposes).

Problem shape: B=8, L=1024, H=1024 (16 heads x 64), MAX_POS=1024.
Sharding: data-parallel over batch -- core b computes batch element b fully.

Math (per batch, per head):
    q = x @ Wq.T + bq ; k, v likewise
    S[l,r] = (q[l]@k[r] + q[l]@de[l-r+1023] + k[r]@de[l-r+1023]) / 8
    P = softmax(S, axis=r);  ctx[l,:] = P[l,:] @ v

Kernel formulation (transposed scores S^T[r,l]):
    - host pre-transposes: xT[j,l], WqT/8, WkT, WvT, de tables; projections
      and QK run in bf16 (PSUM accumulates f32).
    - Toeplitz position terms via banded outer-product matrices in DRAM with
      column-reversed band layout, re-read with a stride-trick AP that
      realizes the per-row diagonal shift. Band blocks pack a head PAIR side
      by side ([128, 2304] fp8e4m3, x64 scaled); partition stride 2303 keeps
      the skew per head.
    - k-term skew tiles land directly as kposT[r',l] and are added into the
      score PSUM by a matmul against a 1/64-scaled identity; q-term tiles
      land as qpos[l',r] and are transposed into the score PSUM two
      128-chunks at a time by fp8 DoubleRow matmuls against a block identity
      ([128, 2, 256], eye/64 blocks) -- out = aq(lb)^T | aq(lb+1)^T.
    - phase B (bands) and phase C (scores/softmax/AV) are software-pipelined
      across head pairs: each emission "round" zips 8 band blocks of pair p
      with 16 rb score units of pair p-1; k-term tiles are loaded
      just-in-time ~3 blocks ahead; scores use chunked [128,512] PSUM tiles
      with per-chunk exp for fine-grained PSUM recycling.
    - softmax without max subtraction (logits bounded); denominator via an
      appended ones-column on v so Z comes out of the AV matmul for free.
    - output produced transposed (ctx*Z | Z rows); host divides+transposes.
"""

import sys

sys.path.insert(0, "/opt/trn_rl_repo")

import numpy as np

import concourse.bass as bass
import concourse.mybir as mybir
import concourse.tile as tile
from concourse import bacc
from concourse.bass_utils import run_bass_kernel_spmd

F32 = mybir.dt.float32
F32R = mybir.dt.float32r
BF16 = mybir.dt.bfloat16
FP8 = mybir.dt.float8e4
BF16_NP = mybir.dt.np(BF16)
FP8_NP = mybir.dt.np(FP8)

B = 8
L = 1024
H = 1024
NH = 16
HD = 64
NB = L // 128
NPAIR = NH // 2
BPITCH = 1152          # per-head band pitch
PPITCH = 2 * BPITCH    # pair-packed block pitch (2304)
BSCALE = 64.0          # band store scale (fp8-friendly)
INV_BSCALE = 1.0 / BSCALE
CHUNKS = [(0, 512), (512, 512), (1024, 128)]

TRACE = False
LAST_RESULTS = None

_CACHE = {}


def _enable_ldw_opt():
    # --enable-ldw-opt=true fails codegen on this kernel's f32r LDWEIGHTS
    # ("InstLdweights is not compatible with LDW optimization"); keep the
    # compiler default.
    return


def _emit(nc, tc, ctx, tensors):
    import contextlib

    xT = tensors["xT"]
    wqT8 = tensors["wqT8"]
    wkT = tensors["wkT"]
    wvT = tensors["wvT"]
    bq8 = tensors["bq8"]
    bk = tensors["bk"]
    bv = tensors["bv"]
    det8 = tensors["det8"]      # de.T / 8     [64, 2048] (k-side band rhs)
    detrev = tensors["detrev"]  # de[::-1].T   [64, 2048] (q-side band rhs)
    outTa = tensors["outTa"]

    ACC = mybir.AluOpType
    AF = mybir.ActivationFunctionType

    # ---------------- persistent pools ----------------
    persist = ctx.enter_context(tc.tile_pool(name="persist", bufs=1))
    qT8_sb = [persist.tile([128, L], BF16, tag=f"qT8_{t}", name=f"qT8_{t}")
              for t in range(NB)]
    kT_sb = [persist.tile([128, L], BF16, tag=f"kT_{t}", name=f"kT_{t}")
             for t in range(NB)]
    vaug_sb = [persist.tile([128, NH * (HD + 1)], BF16, tag=f"vaug_{t}",
                            name=f"vaug_{t}") for t in range(NB)]
    bias_sb = persist.tile([128, 2 * NB], F32, tag="bias")
    det8_sb = persist.tile([128, 2048], BF16, tag="det8")
    detrev_sb = persist.tile([128, 2048], BF16, tag="detrev")

    nc.sync.dma_start(
        out=bias_sb[:, 0:NB],
        in_=bass.AP(tensor=bq8.tensor, offset=0, ap=[[1, 128], [128, NB]]),
    )
    nc.sync.dma_start(
        out=bias_sb[:, NB:2 * NB],
        in_=bass.AP(tensor=bk.tensor, offset=0, ap=[[1, 128], [128, NB]]),
    )
    # de tables replicated on both partition halves (for head-pair packing)
    nc.sync.dma_start(out=det8_sb[0:64, :], in_=det8[:, :])
    nc.sync.dma_start(out=det8_sb[64:128, :], in_=det8[:, :])
    nc.sync.dma_start(out=detrev_sb[0:64, :], in_=detrev[:, :])
    nc.sync.dma_start(out=detrev_sb[64:128, :], in_=detrev[:, :])

    ident2 = tensors["ident2"]
    # block identity for DoubleRow transposes: [128, 2, 256] fp8, eye/64 at
    # [:, 0, 0:128] and [:, 1, 128:256]
    ident2_sb = persist.tile([128, 512], FP8, tag="ident2")
    nc.sync.dma_start(out=ident2_sb, in_=ident2[:, :])
    ident2_ap = bass.AP(tensor=ident2_sb.tensor, offset=ident2_sb.offset,
                        ap=[ident2_sb.ap[0], [256, 2], [1, 256]])

    # DRAM scratch for position bands (pair-packed, column-reversed layout)
    dram = ctx.enter_context(tc.tile_pool(name="dramsc", bufs=1, space="DRAM"))
    aq_band = dram.tile([NPAIR, NB, 128, PPITCH], FP8, tag="aq_band")
    ak_band = dram.tile([NPAIR, NB, 128, PPITCH], FP8, tag="ak_band")

    # ---------------- phase A: projections ----------------
    with contextlib.ExitStack() as phase_a:
        xp = phase_a.enter_context(tc.tile_pool(name="xT", bufs=1))
        xT_sb = [xp.tile([128, L], BF16, tag=f"xT_{t}", name=f"xT_{t}")
                 for t in range(NB)]
        for t in range(NB):
            nc.sync.dma_start(out=xT_sb[t], in_=xT[t * 128:(t + 1) * 128, :])
        bv_sb = xp.tile([128, H], F32, tag="bv")
        nc.sync.dma_start(out=bv_sb, in_=bass.AP(tensor=bv.tensor, offset=0,
                                                 ap=[[0, 128], [1, H]]))

        wp = phase_a.enter_context(tc.tile_pool(name="w", bufs=24))
        pp = phase_a.enter_context(
            tc.tile_pool(name="projps", bufs=3, space="PSUM"))
        for wi, (wten, dst, bias_col) in enumerate(
            [(wqT8, qT8_sb, 0), (wkT, kT_sb, NB)]
        ):
            w_sb = [wp.tile([128, H], BF16, tag="wtile", name="wtile")
                    for _ in range(NB)]
            for jt in range(NB):
                nc.sync.dma_start(out=w_sb[jt],
                                  in_=wten[jt * 128:(jt + 1) * 128, :])
            for ib in range(NB):
                ps = pp.tile([128, L], F32, tag="projps")
                for jt in range(NB):
                    for lc in range(2):
                        nc.tensor.matmul(
                            ps[:, lc * 512:(lc + 1) * 512],
                            lhsT=w_sb[jt][:, ib * 128:(ib + 1) * 128],
                            rhs=xT_sb[jt][:, lc * 512:(lc + 1) * 512],
                            start=(jt == 0),
                            stop=(jt == NB - 1),
                        )
                nc.scalar.activation(
                    out=dst[ib],
                    in_=ps,
                    func=AF.Identity,
                    bias=bias_sb[:, bias_col + ib: bias_col + ib + 1],
                    scale=1.0,
                )

        # V natural [r, i] with ones column per head
        w_sb = [wp.tile([128, H], BF16, tag="wtile", name="wtile")
                for _ in range(NB)]
        for jt in range(NB):
            nc.sync.dma_start(out=w_sb[jt],
                              in_=wvT[jt * 128:(jt + 1) * 128, :])
        for rb in range(NB):
            nc.vector.memset(vaug_sb[rb], 1.0)
            ps = pp.tile([128, H], F32, tag="projps")
            for jt in range(NB):
                for ic in range(2):
                    nc.tensor.matmul(
                        ps[:, ic * 512:(ic + 1) * 512],
                        lhsT=xT_sb[jt][:, rb * 128:(rb + 1) * 128],
                        rhs=w_sb[jt][:, ic * 512:(ic + 1) * 512],
                        start=(jt == 0),
                        stop=(jt == NB - 1),
                    )
            for h in range(NH):
                nc.vector.tensor_tensor(
                    out=vaug_sb[rb][:, h * (HD + 1): h * (HD + 1) + HD],
                    in0=ps[:, h * HD:(h + 1) * HD],
                    in1=bv_sb[:, h * HD:(h + 1) * HD],
                    op=ACC.add,
                )

    # ---------------- fused phase B/C pools ----------------
    bandps = ctx.enter_context(tc.tile_pool(name="bandps", bufs=3,
                                            space="PSUM"))
    sps = ctx.enter_context(tc.tile_pool(name="sps", bufs=3, space="PSUM"))
    ctxps = ctx.enter_context(tc.tile_pool(name="ctxps", bufs=1, space="PSUM"))
    stq = ctx.enter_context(tc.tile_pool(name="stq", bufs=2))
    stk = ctx.enter_context(tc.tile_pool(name="stk", bufs=2))
    aqp = ctx.enter_context(tc.tile_pool(name="aqp", bufs=4))
    kpb = ctx.enter_context(tc.tile_pool(name="kpb", bufs=8))
    prb = ctx.enter_context(tc.tile_pool(name="prb", bufs=3))
    fin = ctx.enter_context(tc.tile_pool(name="fin", bufs=2))

    swp = ctx.enter_context(tc.tile_pool(name="swp", bufs=4))
    swap_tiles = {}

    def emit_swap(pr):
        # partition-swapped copies: rows 64:128 <- rows 0:64 and vice versa,
        # so QK lc0/lc1 can run on disjoint PE row strips concurrently
        qx = swp.tile([128, L], BF16, tag="swp", name="swp")
        kx = swp.tile([128, L], BF16, tag="swp", name="swp")
        for dst, srct in ((qx, qT8_sb[pr]), (kx, kT_sb[pr])):
            nc.sync.dma_start(out=dst[64:128, :], in_=srct[0:64, :])
            nc.sync.dma_start(out=dst[0:64, :], in_=srct[64:128, :])
        swap_tiles[pr] = (qx, kx)

    evict_ctr = [0]

    def evict(out, in_):
        # rotate PSUM evictions: 1/4 ACT, 3/4 DVE (DVE has no STT anymore)
        if evict_ctr[0] % 3 == 0:
            nc.scalar.activation(out=out, in_=in_, func=AF.Copy, scale=BSCALE)
        else:
            nc.vector.tensor_scalar_mul(out, in_, BSCALE)
        evict_ctr[0] += 1

    def gen_B(pr):
        """8 units, blk-major: q-band then k-band outer-products per block."""
        for src_sb, de_sb, band, stpool, stdt in ():
            pass
        for blk in range(NB):
            w0 = 896 - 128 * blk
            for src_sb, de_sb, band, stpool, stdt in (
                (qT8_sb[pr], detrev_sb, aq_band, stq, FP8),
                (kT_sb[pr], det8_sb, ak_band, stk, FP8),
            ):
                stage = stpool.tile([128, PPITCH], stdt, tag="st", name="st")
                for (c0, cw) in CHUNKS:
                    ps_lo = bandps.tile([128, 512], F32, tag="bps",
                                        name="bps")
                    ps_hi = bandps.tile([128, 512], F32, tag="bps",
                                        name="bps")
                    nc.tensor.matmul(
                        ps_lo[:, 0:cw],
                        lhsT=src_sb[0:64, blk * 128:(blk + 1) * 128],
                        rhs=de_sb[0:64, w0 + c0: w0 + c0 + cw],
                        start=True, stop=True, skip_group_check=True,
                    )
                    nc.tensor.matmul(
                        ps_hi[:, 0:cw],
                        lhsT=src_sb[64:128, blk * 128:(blk + 1) * 128],
                        rhs=de_sb[64:128, w0 + c0: w0 + c0 + cw],
                        start=True, stop=True, skip_group_check=True,
                    )
                    evict(stage[:, c0:c0 + cw], ps_lo[:, 0:cw])
                    evict(stage[:, BPITCH + c0:BPITCH + c0 + cw],
                          ps_hi[:, 0:cw])
                nc.sync.dma_start(out=band[pr, blk], in_=stage)
            yield

    def skew_ap(band, pr, blk, hcol):
        base = band[pr, blk, :, :]
        return bass.AP(
            tensor=base.tensor,
            offset=base.offset + hcol * BPITCH + 127,
            ap=[[PPITCH - 1, 128], [1, L]],
        )

    aq_tiles = {}    # h -> [128, NB*L] fp8 skew tiles (qpos[l', r] per lb)
    kpt_tiles = {}   # (h, rb) -> k-term skew tile

    def emit_reads(pr, blk):
        """After B(pr) wrote block blk: q-skew reads for both heads."""
        h0, h1 = 2 * pr, 2 * pr + 1
        if blk == 0:
            aq_tiles[h0] = aqp.tile([128, NB * L], FP8, tag="aqp", name="aqp")
            aq_tiles[h1] = aqp.tile([128, NB * L], FP8, tag="aqp", name="aqp")
        for hcol in (0, 1):
            t = aq_tiles[2 * pr + hcol]
            nc.sync.dma_start(out=t[:, blk * L:(blk + 1) * L],
                              in_=skew_ap(aq_band, pr, blk, hcol))

    def kpt_loads_for(rnd, blk):
        """JIT schedule: load (pair, h, rb) k-term tiles ~3 blks ahead."""
        out = []
        if blk == 0:
            out = [(rnd - 1, 0, 6), (rnd - 1, 0, 7)]
        elif blk <= 4:
            out = [(rnd - 1, 1, 2 * (blk - 1)), (rnd - 1, 1, 2 * blk - 1)]
        else:
            out = [(rnd, 0, 2 * (blk - 5)), (rnd, 0, 2 * blk - 9)]
        return [(p, h, rb) for (p, h, rb) in out if 0 <= p < NPAIR]

    def emit_kpt_loads(rnd, blk):
        for (p, hcol, rb) in kpt_loads_for(rnd, blk):
            t = kpb.tile([128, L], FP8, tag="kpt", name="kpt")
            nc.sync.dma_start(out=t, in_=skew_ap(ak_band, p, rb, hcol))
            kpt_tiles[(2 * p + hcol, rb)] = t

    def emit_av(h, ctx_ps, pv_rb, pv_p):
        for lc in range(2):
            nc.tensor.matmul(
                ctx_ps[:, lc * 512:(lc + 1) * 512],
                lhsT=vaug_sb[pv_rb][:, h * (HD + 1):(h + 1) * (HD + 1)],
                rhs=pv_p[:, lc * 512:(lc + 1) * 512],
                start=(pv_rb == 0), stop=(pv_rb == NB - 1),
                skip_group_check=True,
            )

    def gen_C(pr):
        """16 units: 8 rb units per head; AV software-pipelined by one rb."""
        for h in (2 * pr, 2 * pr + 1):
            hrow = (h % 2) * 64
            aq = aq_tiles[h]
            ctx_ps = ctxps.tile([HD + 1, L], F32, tag="ctxps", name="ctxps")
            prev = None  # (odd rb, p_pair tile)
            qx_sb, kx_sb = swap_tiles[pr]
            xrow = 64 - hrow
            for rb in range(NB):
                kpt = kpt_tiles.pop((h, rb))
                p_sb = prb.tile([128, L], BF16, tag="p", name="p")
                # paired QK: lc0 on this head's home strip, lc1 on the other
                # strip via the partition-swapped copies -- they run
                # concurrently on disjoint PE row strips
                s_chunks = [sps.tile([128, 512], F32, tag="sps", name="sps")
                            for _ in range(2)]
                nc.tensor.matmul(
                    s_chunks[0],
                    lhsT=kT_sb[pr][hrow:hrow + 64, rb * 128:(rb + 1) * 128],
                    rhs=qT8_sb[pr][hrow:hrow + 64, 0:512],
                    start=True, stop=False, skip_group_check=True,
                )
                nc.tensor.matmul(
                    s_chunks[1],
                    lhsT=kx_sb[xrow:xrow + 64, rb * 128:(rb + 1) * 128],
                    rhs=qx_sb[xrow:xrow + 64, 512:1024],
                    start=True, stop=False, skip_group_check=True,
                )
                for lc in range(2):
                    s_ps = s_chunks[lc]
                    for j in range(2):
                        lb = lc * 4 + 2 * j
                        # transpose TWO q-term chunks into score PSUM via the
                        # (1/64-scaled) block identity in fp8 DoubleRow mode:
                        # out[:, 256] = aq(lb)^T | aq(lb+1)^T
                        lhsT = bass.AP(
                            tensor=aq.tensor,
                            offset=aq.offset + lb * L + rb * 128,
                            ap=[aq.ap[0], [L, 2], [1, 128]],
                        )
                        nc.tensor.matmul(
                            s_ps[:, 256 * j:256 * (j + 1)],
                            lhsT=lhsT,
                            rhs=ident2_ap,
                            start=False, stop=(lc == 0 and j == 1),
                            perf_mode=mybir.MatmulPerfMode.DoubleRow,
                            skip_group_check=True,
                        )
                    # k-term: chunk 0 via DVE (tensor is the bottleneck),
                    # chunk 1 via the (1/64-scaled) identity on the PE
                    if lc == 0:
                        nc.vector.scalar_tensor_tensor(
                            out=s_ps, in0=kpt[:, 0:512],
                            scalar=INV_BSCALE, in1=s_ps,
                            op0=ACC.mult, op1=ACC.add,
                        )
                    else:
                        nc.tensor.matmul(
                            s_ps,
                            lhsT=ident2_sb[:, 0:128],
                            rhs=kpt[:, 512:1024],
                            start=False, stop=True,
                            skip_group_check=True,
                        )
                    nc.scalar.activation(out=p_sb[:, lc * 512:(lc + 1) * 512],
                                         in_=s_ps, func=AF.Exp)
                if prev is not None:
                    pv_rb, pv_p = prev
                    emit_av(h, ctx_ps, pv_rb, pv_p)
                    prev = None
                prev = (rb, p_sb)
                yield
            # drain AV(rb=7) + finalize head
            pv_rb, pv_p = prev
            emit_av(h, ctx_ps, pv_rb, pv_p)
            o_sb = fin.tile([HD + 1, L], F32, tag="osb", name="osb")
            nc.vector.tensor_copy(out=o_sb, in_=ctx_ps)
            nc.sync.dma_start(
                out=outTa[h * (HD + 1):(h + 1) * (HD + 1), :], in_=o_sb)

    # ---------------- pipeline driver ----------------
    # round rnd: B(rnd) band production + per-block reads, zipped with
    # C(rnd-1); round NPAIR is the drain (no band work).
    emit_swap(0)
    for rnd in range(NPAIR + 1):
        bg = gen_B(rnd) if rnd < NPAIR else None
        cg = gen_C(rnd - 1) if rnd >= 1 else None
        if 0 < rnd < NPAIR:
            emit_swap(rnd)
        for blk in range(NB):
            if bg is not None:
                next(bg)
            emit_kpt_loads(rnd, blk)
            if bg is not None:
                emit_reads(rnd, blk)
            if cg is not None:
                next(cg)
                next(cg)
        if cg is not None:
            for _ in cg:   # emit the tail (last AV + finalize of odd head)
                pass


def build_nc():
    if "nc" in _CACHE:
        return _CACHE["nc"]
    import contextlib
    _enable_ldw_opt()

    nc = bacc.Bacc("TRN2", target_bir_lowering=False, debug=False)
    tensors = {
        "xT": nc.dram_tensor("xT", [H, L], BF16, kind="ExternalInput").ap(),
        "wqT8": nc.dram_tensor("wqT8", [H, H], BF16,
                               kind="ExternalInput").ap(),
        "wkT": nc.dram_tensor("wkT", [H, H], BF16, kind="ExternalInput").ap(),
        "wvT": nc.dram_tensor("wvT", [H, H], BF16, kind="ExternalInput").ap(),
        "bq8": nc.dram_tensor("bq8", [H], F32, kind="ExternalInput").ap(),
        "bk": nc.dram_tensor("bk", [H], F32, kind="ExternalInput").ap(),
        "bv": nc.dram_tensor("bv", [H], F32, kind="ExternalInput").ap(),
        "det8": nc.dram_tensor("det8", [HD, 2048], BF16,
                               kind="ExternalInput").ap(),
        "detrev": nc.dram_tensor("detrev", [HD, 2048], BF16,
                                 kind="ExternalInput").ap(),
        "ident2": nc.dram_tensor("ident2", [128, 512], FP8,
                                 kind="ExternalInput").ap(),
        "outTa": nc.dram_tensor("outTa", [NH * (HD + 1), L], F32,
                                kind="ExternalOutput").ap(),
    }
    with contextlib.ExitStack() as ctx:
        tc = ctx.enter_context(tile.TileContext(nc))
        _emit(nc, tc, ctx, tensors)
    nc.compile()
    _CACHE["nc"] = nc
    return nc


def _host_inputs(hidden_states, attention_mask, Wq, bq, Wk, bk, Wv, bv,
                 dist_emb):
    f32 = np.float32
    de = np.ascontiguousarray(dist_emb, dtype=f32)
    pad = np.zeros((HD, 1), np.float32)
    det8 = np.ascontiguousarray(
        np.concatenate([de.T / 8.0, pad], axis=1)).astype(BF16_NP)
    detrev = np.ascontiguousarray(
        np.concatenate([de[::-1].T, pad], axis=1)).astype(BF16_NP)
    ident2 = np.zeros((128, 512), np.float32)
    ident2[np.arange(128), np.arange(128)] = 1.0 / BSCALE
    ident2[np.arange(128), 384 + np.arange(128)] = 1.0 / BSCALE
    base = {
        "wqT8": np.ascontiguousarray(Wq.astype(f32).T / 8.0).astype(BF16_NP),
        "wkT": np.ascontiguousarray(Wk.astype(f32).T).astype(BF16_NP),
        "wvT": np.ascontiguousarray(Wv.astype(f32).T).astype(BF16_NP),
        "bq8": np.ascontiguousarray(bq, dtype=f32) / 8.0,
        "bk": np.ascontiguousarray(bk, dtype=f32),
        "bv": np.ascontiguousarray(bv, dtype=f32),
        "det8": det8, "detrev": detrev,
        "ident2": ident2.astype(FP8_NP),
    }
    in_maps = []
    for b in range(B):
        m = dict(base)
        m["xT"] = np.ascontiguousarray(
            hidden_states[b].astype(f32).T).astype(BF16_NP)
        in_maps.append(m)
    return in_maps


def kernel(**inputs):
    global LAST_RESULTS
    nc = build_nc()
    in_maps = _host_inputs(**{k: np.asarray(v) for k, v in inputs.items()})
    res = run_bass_kernel_spmd(nc, in_maps, core_ids=list(range(B)),
                               trace=TRACE)
    LAST_RESULTS = res
    out = np.empty((B, L, H), np.float32)
    for b in range(B):
        a = res.results[b]["outTa"].reshape(NH, HD + 1, L)
        ctx = a[:, :HD, :] / a[:, HD:HD + 1, :]      # [NH, HD, L]
        out[b] = ctx.transpose(2, 0, 1).reshape(L, H)
    return out


if __name__ == "__main__":
    rng = np.random.default_rng(0)
    demo = {
        "hidden_states": rng.standard_normal((B, L, H), dtype=np.float32),
        "attention_mask": np.zeros((B, 1, 1, L), np.float32),
        "Wq": rng.standard_normal((H, H), dtype=np.float32) * 0.02,
        "bq": np.zeros(H, np.float32),
        "Wk": rng.standard_normal((H, H), dtype=np.float32) * 0.02,
        "bk": np.zeros(H, np.float32),
        "Wv": rng.standard_normal((H, H), dtype=np.float32) * 0.02,
        "bv": np.zeros(H, np.float32),
        "dist_emb": rng.standard_normal((2047, HD), dtype=np.float32) * 0.02,
    }
    out = kernel(**demo)
    print(out.shape, out.dtype)
